# revision 6
# baseline (speedup 1.0000x reference)
"""Trainium2 Bass kernel for batched MAPK-cascade ODE step (nn_ModelStep_57715770523995).

Integrates dy/dt = S @ v(y; c) for B=32768 independent trajectories from t to
t+deltaT with fixed-step RK4 (NSTEPS steps), sharded batch-parallel across 8
NeuronCores (4096 trajectories per core).

Per-core layout: [128 partitions, 256 free] = 16 trajectory groups x 8 state
slots x 256 trajectories. Slots hold replicated species [M, Y, T, 1, Y, T, P, M]
so that:
  - den1/den2 (shared Michaelis-Menten denominators) are per-group partition
    sums of slot-wise products -> one block-diagonal fp32 matmul on the tensor
    engine (the "+1" rides in the constant slot 3),
  - the 7 reaction rates are slot-wise products x_s * a_s * (1/den_s) on the
    vector engine,
  - dy = S@v is another block-diagonal +-1 matmul; RK4 stage combinations are
    scalar_tensor_tensor ops.

Raw bass blocks with explicit semaphores (one wait per instruction - the
toolchain's TT/DMA instruction encodings hold a single sync wait).
"""

from collections import defaultdict
from contextlib import ExitStack

import numpy as np

NCORES = 8
B = 32768
PER_CORE = B // NCORES          # 4096
G = 16                          # trajectory groups per core
SLOTS = 8
F = PER_CORE // G               # 256 trajectories per group
NSTEPS = 3
RECIP_MODE = "exact"            # "exact" (DVE reciprocal) | "nr" (exp/ln + 1 Newton)

# slot -> y column (M=0, Y=1, P=2, T=3); -1 means constant 1.0
SLOT_SRC = [0, 1, 3, -1, 1, 3, 2, 0]


# --------------------------------------------------------------------------
# host-side constants / weights
# --------------------------------------------------------------------------
def _derived_constants(c):
    MEK, MKP3, cell = c[:, 0], c[:, 1], c[:, 17]
    d = {}
    d["r2"] = (c[:, 2] + c[:, 6]) / (c[:, 2] * c[:, 6])
    d["ic4"] = 1.0 / c[:, 4]
    d["ic8"] = 1.0 / c[:, 8]
    d["ic10"] = 1.0 / c[:, 10]
    d["ic12"] = 1.0 / c[:, 12]
    d["ic14"] = 1.0 / c[:, 14]
    d["ic16"] = 1.0 / c[:, 16]
    d["a1"] = cell * c[:, 3] * MEK / c[:, 2]
    d["a2"] = cell * c[:, 5] * MEK / c[:, 4]
    d["a3"] = cell * c[:, 7] * MEK / c[:, 6]
    d["a4"] = cell * c[:, 9] * MEK / c[:, 8]
    d["a5"] = cell * c[:, 11] * MKP3 / c[:, 10]
    d["a6"] = cell * c[:, 13] * MKP3 / c[:, 12]
    d["a7"] = cell * c[:, 15] * MKP3 / c[:, 14]
    return d


def _build_core_inputs(yk, ck, h):
    """yk [4096,4] f32, ck [4096,18] f32 -> X8, AB, A8 as [128, F] f32."""
    d = _derived_constants(ck.astype(np.float64))

    def gf(a):
        return a.reshape(G, F)

    y_g = yk.reshape(G, F, 4)
    X8 = np.empty((G, SLOTS, F), np.float32)
    for s, src in enumerate(SLOT_SRC):
        X8[:, s, :] = 1.0 if src < 0 else y_g[:, :, src]
    ab_slots = [d["r2"], d["ic4"], d["ic8"], None,
                d["ic14"], d["ic12"], d["ic10"], d["ic16"]]
    AB = np.empty((G, SLOTS, F), np.float32)
    for s, v in enumerate(ab_slots):
        AB[:, s, :] = 1.0 if v is None else gf(v).astype(np.float32)
    a_slots = [d["a1"], d["a2"], d["a4"], None, d["a7"], d["a6"], d["a5"], d["a3"]]
    A8 = np.empty((G, SLOTS, F), np.float32)
    for s, v in enumerate(a_slots):
        A8[:, s, :] = 0.0 if v is None else (h * gf(v)).astype(np.float32)
    return X8.reshape(128, F), AB.reshape(128, F), A8.reshape(128, F)


def _build_weights():
    """W[k, m] = coefficient of input partition k for output partition m (lhsT)."""
    wden_blk = np.zeros((SLOTS, SLOTS), np.float32)
    for m in (0, 1, 2, 3, 7):            # den1 consumers
        for k in (0, 1, 2, 3):           # den1 = Z0+Z1+Z2+Z3 (Z3 == 1)
            wden_blk[k, m] = 1.0
    for m in (4, 5, 6):                  # den2 consumers
        for k in (3, 4, 5, 6, 7):        # den2 = Z4+..+Z7 + Z3(=1)
            wden_blk[k, m] = 1.0
    # V slots [v1, v2, v4, 0, v7, v6, v5, v3]; out roles [dM,dY,dT,0,dY,dT,dP,dM]
    dy_coef = {
        "dM": {0: -1, 7: -1, 5: +1, 4: +1},
        "dY": {0: +1, 1: -1, 4: -1},
        "dP": {1: +1, 2: +1, 6: -1},
        "dT": {7: +1, 2: -1, 6: +1, 5: -1},
    }
    roles = ["dM", "dY", "dT", None, "dY", "dT", "dP", "dM"]
    wk_blk = np.zeros((SLOTS, SLOTS), np.float32)
    for m, role in enumerate(roles):
        if role is None:
            continue
        for k, coef in dy_coef[role].items():
            wk_blk[k, m] = coef

    def blockdiag(blk):
        W = np.zeros((128, 128), np.float32)
        for g in range(G):
            W[g * 8:(g + 1) * 8, g * 8:(g + 1) * 8] = blk
        return W

    return blockdiag(wden_blk), blockdiag(wk_blk), blockdiag(2.0 * wk_blk)


def _extract_y(y8):
    """y8 [128, F] -> yk [4096, 4]"""
    a = y8.reshape(G, SLOTS, F)
    out = np.empty((G, F, 4), y8.dtype)
    out[:, :, 0] = a[:, 0, :]
    out[:, :, 1] = a[:, 1, :]
    out[:, :, 2] = a[:, 6, :]
    out[:, :, 3] = a[:, 2, :]
    return out.reshape(PER_CORE, 4)


# --------------------------------------------------------------------------
# raw-bass multi-engine scheduler (one sync-wait per instruction)
# --------------------------------------------------------------------------
class _Op:
    __slots__ = ("engine", "fn", "reads", "writes", "inc", "deps",
                 "need_inc", "inc_count")

    def __init__(self, engine, fn, reads, writes, inc):
        self.engine = engine
        self.fn = fn
        self.reads = list(reads)
        self.writes = list(writes)
        self.inc = inc
        self.deps = []
        self.need_inc = False
        self.inc_count = None


class _Prog:
    def __init__(self):
        self.ops = []
        self.writer = {}
        self.readers = defaultdict(list)

    def op(self, engine, fn, reads=(), writes=(), inc=1):
        o = _Op(engine, fn, reads, writes, inc)
        for b in o.reads:
            w = self.writer.get(b)
            if w is not None:
                o.deps.append(w)
        for b in o.writes:
            w = self.writer.get(b)
            if w is not None:
                o.deps.append(w)          # WAW
            for r in self.readers[b]:
                o.deps.append(r)          # WAR
        for b in o.reads:
            self.readers[b].append(o)
        for b in o.writes:
            self.writer[b] = o
            self.readers[b] = []
        self.ops.append(o)
        return o

    def streams(self):
        for o in self.ops:
            if o.engine == "sync":
                o.need_inc = True
            for d in o.deps:
                if d.engine != o.engine:
                    d.need_inc = True
        counts = defaultdict(int)
        for o in self.ops:
            if o.need_inc:
                counts[o.engine] += o.inc
            o.inc_count = counts[o.engine]
        streams = defaultdict(list)
        waited = defaultdict(lambda: defaultdict(int))
        for o in self.ops:
            waits = []
            for d in o.deps:
                if d.engine == o.engine:
                    continue
                thr = d.inc_count
                if waited[o.engine][d.engine] < thr:
                    waits.append((d.engine, thr))
                    waited[o.engine][d.engine] = thr
            streams[o.engine].append((waits, o))
        return streams, dict(counts)


# --------------------------------------------------------------------------
# bass program
# --------------------------------------------------------------------------
def _build_nc(nsteps=None):
    if nsteps is None:
        nsteps = NSTEPS
    import concourse.bass as bass
    import concourse.mybir as mybir

    fp32 = mybir.dt.float32
    Alu = mybir.AluOpType
    Act = mybir.ActivationFunctionType

    nc = bass.Bass(
        "TRN2",
        target_bir_lowering=False,
        debug=False,
        enable_asserts=False,
        num_devices=NCORES,
    )

    cin_in = nc.dram_tensor("cin", [128, 3 * F], fp32, kind="ExternalInput").ap()
    win_in = nc.dram_tensor("win", [128, 384], fp32, kind="ExternalInput").ap()
    y8_out = nc.dram_tensor("y8", [128, F], fp32, kind="ExternalOutput").ap()

    ctx = ExitStack()
    sb = lambda name, shape: ctx.enter_context(
        nc.sbuf_tensor(name, shape, fp32)).ap()
    pst = lambda name: ctx.enter_context(
        nc.psum_tensor(name, [128, F], fp32)).ap()

    cin_t = sb("cin_t", [128, 3 * F])
    win_t = sb("win_t", [128, 384])
    ab_t = cin_t[:, F:2 * F]
    a8_t = cin_t[:, 2 * F:3 * F]
    wden_t = win_t[:, 0:128]
    wk1_t = win_t[:, 128:256]
    wk2_t = win_t[:, 256:384]
    x_t = [sb("x0_t", [128, F]), sb("x1_t", [128, F])]
    xs_t = sb("xs_t", [128, F])
    z_t = [sb("z0_t", [128, F]), sb("z1_t", [128, F])]
    t_t = sb("t_t", [128, F])
    v_t = [sb("v0_t", [128, F]), sb("v1_t", [128, F])]
    idr_t = sb("idr_t", [128, F])
    ln_t = sb("ln_t", [128, F])
    q_t = sb("q_t", [128, F])
    r_t = sb("r_t", [128, F])
    den_ps = pst("den_ps")
    k8_ps = pst("k8_ps")
    ksum_ps = pst("ksum_ps")

    p = _Prog()
    p.op("sync", lambda e: e.dma_start(out=cin_t, in_=cin_in),
         reads=(), writes=["x0raw", "ab", "a8"], inc=16)
    p.op("sync", lambda e: e.dma_start(out=win_t, in_=win_in),
         reads=(), writes=["wden", "wk1", "wk2"], inc=16)
    # state must live in its own ping-pong buffer: copy out of cin
    p.op("vector", lambda e: e.tensor_copy(x_t[0], cin_t[:, 0:F]),
         reads=["x0raw"], writes=["x0"])

    stage_c = [0.5, 0.5, 1.0]
    ksum_w = [("wk1", None), ("wk2", None), ("wk2", None), ("wk1", None)]
    wmap = {"wk1": wk1_t, "wk2": wk2_t}

    for step in range(nsteps):
        xb = f"x{step % 2}"
        xnb = f"x{(step + 1) % 2}"
        x_ap = x_t[step % 2]
        xn_ap = x_t[(step + 1) % 2]
        for s in range(4):
            cur_b = xb if s == 0 else "xs"
            cur_ap = x_ap if s == 0 else xs_t
            zb = s % 2
            vb = s % 2
            p.op("vector",
                 lambda e, o=z_t[zb], a=cur_ap: e.tensor_tensor(o, a, ab_t, Alu.mult),
                 reads=[cur_b, "ab"], writes=[f"z{zb}"])
            p.op("tensor",
                 lambda e, z=z_t[zb]: e.matmul(den_ps, wden_t, z, start=True, stop=True),
                 reads=[f"z{zb}", "wden"], writes=["den"])
            p.op("vector",
                 lambda e, a=cur_ap: e.tensor_tensor(t_t, a, a8_t, Alu.mult),
                 reads=[cur_b, "a8"], writes=["t"])
            if RECIP_MODE == "exact":
                p.op("vector", lambda e: e.reciprocal(idr_t, den_ps),
                     reads=["den"], writes=["idr"])
            else:
                p.op("scalar", lambda e: e.activation(ln_t, den_ps, Act.Ln),
                     reads=["den"], writes=["ln"])
                p.op("scalar",
                     lambda e: e.activation(q_t, ln_t, Act.Exp, scale=-1.0),
                     reads=["ln"], writes=["id0"])
                # one Newton pass: idr = id0 * (2 - den*id0)
                p.op("vector", lambda e: e.tensor_tensor(r_t, den_ps, q_t, Alu.mult),
                     reads=["den", "id0"], writes=["r"])
                p.op("vector",
                     lambda e: e.tensor_scalar(r_t, r_t, -1.0, 2.0, Alu.mult, Alu.add),
                     reads=["r"], writes=["r"])
                p.op("vector", lambda e: e.tensor_tensor(idr_t, q_t, r_t, Alu.mult),
                     reads=["id0", "r"], writes=["idr"])
            p.op("vector",
                 lambda e, o=v_t[vb]: e.tensor_tensor(o, t_t, idr_t, Alu.mult),
                 reads=["t", "idr"], writes=[f"v{vb}"])
            if s < 3:
                p.op("tensor",
                     lambda e, v=v_t[vb]: e.matmul(k8_ps, wk1_t, v, start=True, stop=True),
                     reads=[f"v{vb}", "wk1"], writes=["k8"])
            kw = ksum_w[s][0]
            p.op("tensor",
                 lambda e, w=wmap[kw], v=v_t[vb], st=(s == 0), sp=(s == 3): e.matmul(
                     ksum_ps, w, v, start=st, stop=sp),
                 reads=[f"v{vb}", kw] + ([] if s == 0 else ["ksum"]),
                 writes=["ksum"])
            if s < 3:
                p.op("vector",
                     lambda e, c=stage_c[s], x=x_ap: e.scalar_tensor_tensor(
                         xs_t, k8_ps, float(c), x, Alu.mult, Alu.add),
                     reads=["k8", xb], writes=["xs"])
        p.op("vector",
             lambda e, x=x_ap, xn=xn_ap: e.scalar_tensor_tensor(
                 xn, ksum_ps, 1.0 / 6.0, x, Alu.mult, Alu.add),
             reads=["ksum", xb], writes=[xnb])

    xfb = f"x{nsteps % 2}"
    p.op("sync", lambda e: e.dma_start(out=y8_out, in_=x_t[nsteps % 2]),
         reads=[xfb], writes=["out"], inc=16)

    streams, counts = p.streams()

    with nc.Block() as block, ExitStack() as semctx:
        sems = {n: semctx.enter_context(nc.semaphore(f"sem_{n}"))
                for n in streams}

        def make_body(ename):
            stream = streams[ename]
            total = counts.get(ename, 0)

            def body(eng):
                for waits, o in stream:
                    for (peng, thr) in waits:
                        eng.wait_ge(sems[peng], thr)
                    inst = o.fn(eng)
                    if o.need_inc:
                        inst.then_inc(sems[o.engine], o.inc)
                if ename == "sync":
                    eng.wait_ge(sems["sync"], total)
            return body

        for ename in streams:
            getattr(block, ename)(make_body(ename))

    ctx.close()
    return nc


_CACHE = {}


def _get_nc():
    if "nc" not in _CACHE:
        _CACHE["nc"] = _build_nc()
    return _CACHE["nc"]


def kernel(y, w, c, t, deltaT):
    from concourse.bass_utils import run_bass_kernel_spmd

    y = np.asarray(y, dtype=np.float32)
    w_np = np.asarray(w, dtype=np.float32)
    c_np = np.asarray(c, dtype=np.float32)
    t_f = np.float32(np.asarray(t))
    dT = np.float32(np.asarray(deltaT))
    h = float(dT) / NSTEPS

    wden, wk1, wk2 = _build_weights()
    win = np.ascontiguousarray(np.concatenate([wden, wk1, wk2], axis=1))
    in_maps = []
    for k in range(NCORES):
        sl = slice(k * PER_CORE, (k + 1) * PER_CORE)
        X8, AB, A8 = _build_core_inputs(y[sl], c_np[sl], h)
        in_maps.append({
            "cin": np.ascontiguousarray(np.concatenate([X8, AB, A8], axis=1)),
            "win": win,
        })

    nc = _get_nc()
    res = run_bass_kernel_spmd(nc, in_maps, list(range(NCORES)))
    y_new = np.concatenate(
        [_extract_y(res.results[k]["y8"]) for k in range(NCORES)], axis=0)
    return (y_new, w_np, c_np, np.float32(t_f + dT))


# revision 7
# speedup vs baseline: 1.5751x; 1.5751x over previous
"""Trainium2 Bass kernel for batched MAPK-cascade ODE step (nn_ModelStep_57715770523995).

Integrates dy/dt = S @ v(y; c) for B=32768 independent trajectories from t to
t+deltaT with fixed-step RK4 (NSTEPS steps), sharded batch-parallel across 8
NeuronCores (4096 trajectories per core).

Per-core layout: [128 partitions, 256 free] = 16 trajectory groups x 8 state
slots x 256 trajectories. Slots hold replicated species [M, Y, T, 1, Y, T, P, M]
so that:
  - den1/den2 (shared Michaelis-Menten denominators) are per-group partition
    sums of slot-wise products -> one block-diagonal fp32 matmul on the tensor
    engine (the "+1" rides in the constant slot 3),
  - the 7 reaction rates are slot-wise products x_s * a_s * (1/den_s) on the
    vector engine,
  - dy = S@v is another block-diagonal +-1 matmul; RK4 stage combinations are
    scalar_tensor_tensor ops.

Raw bass blocks with explicit semaphores (one wait per instruction - the
toolchain's TT/DMA instruction encodings hold a single sync wait).
"""

from collections import defaultdict
from contextlib import ExitStack

import numpy as np

NCORES = 8
B = 32768
PER_CORE = B // NCORES          # 4096
G = 16                          # trajectory groups per core
SLOTS = 8
F = PER_CORE // G               # 256 trajectories per group
NSTEPS = 3
RECIP_MODE = "exact"            # "exact" (DVE reciprocal) | "nr" (exp/ln + 1 Newton)

# slot -> y column (M=0, Y=1, P=2, T=3); -1 means constant 1.0
SLOT_SRC = [0, 1, 3, -1, 1, 3, 2, 0]


# --------------------------------------------------------------------------
# host-side constants / weights
# --------------------------------------------------------------------------
def _derived_constants(c):
    MEK, MKP3, cell = c[:, 0], c[:, 1], c[:, 17]
    d = {}
    d["r2"] = (c[:, 2] + c[:, 6]) / (c[:, 2] * c[:, 6])
    d["ic4"] = 1.0 / c[:, 4]
    d["ic8"] = 1.0 / c[:, 8]
    d["ic10"] = 1.0 / c[:, 10]
    d["ic12"] = 1.0 / c[:, 12]
    d["ic14"] = 1.0 / c[:, 14]
    d["ic16"] = 1.0 / c[:, 16]
    d["a1"] = cell * c[:, 3] * MEK / c[:, 2]
    d["a2"] = cell * c[:, 5] * MEK / c[:, 4]
    d["a3"] = cell * c[:, 7] * MEK / c[:, 6]
    d["a4"] = cell * c[:, 9] * MEK / c[:, 8]
    d["a5"] = cell * c[:, 11] * MKP3 / c[:, 10]
    d["a6"] = cell * c[:, 13] * MKP3 / c[:, 12]
    d["a7"] = cell * c[:, 15] * MKP3 / c[:, 14]
    return d


def _build_core_inputs(yk, ck, h):
    """yk [4096,4] f32, ck [4096,18] f32 -> X8, AB, A8 as [128, F] f32."""
    d = _derived_constants(ck.astype(np.float64))

    def gf(a):
        return a.reshape(G, F)

    y_g = yk.reshape(G, F, 4)
    X8 = np.empty((G, SLOTS, F), np.float32)
    for s, src in enumerate(SLOT_SRC):
        X8[:, s, :] = 1.0 if src < 0 else y_g[:, :, src]
    ab_slots = [d["r2"], d["ic4"], d["ic8"], None,
                d["ic14"], d["ic12"], d["ic10"], d["ic16"]]
    AB = np.empty((G, SLOTS, F), np.float32)
    for s, v in enumerate(ab_slots):
        AB[:, s, :] = 1.0 if v is None else gf(v).astype(np.float32)
    a_slots = [d["a1"], d["a2"], d["a4"], None, d["a7"], d["a6"], d["a5"], d["a3"]]
    A8 = np.empty((G, SLOTS, F), np.float32)
    for s, v in enumerate(a_slots):
        A8[:, s, :] = 0.0 if v is None else (h * gf(v)).astype(np.float32)
    return X8.reshape(128, F), AB.reshape(128, F), A8.reshape(128, F)


def _build_weights():
    """W[k, m] = coefficient of input partition k for output partition m (lhsT)."""
    wden_blk = np.zeros((SLOTS, SLOTS), np.float32)
    for m in (0, 1, 2, 3, 7):            # den1 consumers
        for k in (0, 1, 2, 3):           # den1 = Z0+Z1+Z2+Z3 (Z3 == 1)
            wden_blk[k, m] = 1.0
    for m in (4, 5, 6):                  # den2 consumers
        for k in (3, 4, 5, 6, 7):        # den2 = Z4+..+Z7 + Z3(=1)
            wden_blk[k, m] = 1.0
    # V slots [v1, v2, v4, 0, v7, v6, v5, v3]; out roles [dM,dY,dT,0,dY,dT,dP,dM]
    dy_coef = {
        "dM": {0: -1, 7: -1, 5: +1, 4: +1},
        "dY": {0: +1, 1: -1, 4: -1},
        "dP": {1: +1, 2: +1, 6: -1},
        "dT": {7: +1, 2: -1, 6: +1, 5: -1},
    }
    roles = ["dM", "dY", "dT", None, "dY", "dT", "dP", "dM"]
    wk_blk = np.zeros((SLOTS, SLOTS), np.float32)
    for m, role in enumerate(roles):
        if role is None:
            continue
        for k, coef in dy_coef[role].items():
            wk_blk[k, m] = coef

    def blockdiag(blk):
        W = np.zeros((128, 128), np.float32)
        for g in range(G):
            W[g * 8:(g + 1) * 8, g * 8:(g + 1) * 8] = blk
        return W

    return blockdiag(wden_blk), blockdiag(wk_blk), blockdiag(2.0 * wk_blk)


def _extract_y(y8):
    """y8 [128, F] -> yk [4096, 4]"""
    a = y8.reshape(G, SLOTS, F)
    out = np.empty((G, F, 4), y8.dtype)
    out[:, :, 0] = a[:, 0, :]
    out[:, :, 1] = a[:, 1, :]
    out[:, :, 2] = a[:, 6, :]
    out[:, :, 3] = a[:, 2, :]
    return out.reshape(PER_CORE, 4)


# --------------------------------------------------------------------------
# raw-bass multi-engine scheduler (one sync-wait per instruction)
# --------------------------------------------------------------------------
class _Op:
    __slots__ = ("engine", "fn", "reads", "writes", "inc", "deps",
                 "need_inc", "inc_count")

    def __init__(self, engine, fn, reads, writes, inc):
        self.engine = engine
        self.fn = fn
        self.reads = list(reads)
        self.writes = list(writes)
        self.inc = inc
        self.deps = []
        self.need_inc = False
        self.inc_count = None


class _Prog:
    def __init__(self):
        self.ops = []
        self.writer = {}
        self.readers = defaultdict(list)

    def op(self, engine, fn, reads=(), writes=(), inc=1):
        o = _Op(engine, fn, reads, writes, inc)
        for b in o.reads:
            w = self.writer.get(b)
            if w is not None:
                o.deps.append(w)
        for b in o.writes:
            w = self.writer.get(b)
            if w is not None:
                o.deps.append(w)          # WAW
            for r in self.readers[b]:
                o.deps.append(r)          # WAR
        for b in o.reads:
            self.readers[b].append(o)
        for b in o.writes:
            self.writer[b] = o
            self.readers[b] = []
        self.ops.append(o)
        return o

    def streams(self):
        for o in self.ops:
            if o.engine == "sync":
                o.need_inc = True
            for d in o.deps:
                if d.engine != o.engine:
                    d.need_inc = True
        counts = defaultdict(int)
        for o in self.ops:
            if o.need_inc:
                counts[o.engine] += o.inc
            o.inc_count = counts[o.engine]
        streams = defaultdict(list)
        waited = defaultdict(lambda: defaultdict(int))
        for o in self.ops:
            waits = []
            for d in o.deps:
                if d.engine == o.engine:
                    continue
                thr = d.inc_count
                if waited[o.engine][d.engine] < thr:
                    waits.append((d.engine, thr))
                    waited[o.engine][d.engine] = thr
            streams[o.engine].append((waits, o))
        return streams, dict(counts)


# --------------------------------------------------------------------------
# bass program
# --------------------------------------------------------------------------
def _build_nc(nsteps=None):
    if nsteps is None:
        nsteps = NSTEPS
    import concourse.bass as bass
    import concourse.mybir as mybir

    fp32 = mybir.dt.float32
    Alu = mybir.AluOpType
    Act = mybir.ActivationFunctionType

    nc = bass.Bass(
        "TRN2",
        target_bir_lowering=False,
        debug=False,
        enable_asserts=False,
        num_devices=NCORES,
    )

    cin_in = nc.dram_tensor("cin", [128, 3 * F], fp32, kind="ExternalInput").ap()
    win_in = nc.dram_tensor("win", [128, 384], fp32, kind="ExternalInput").ap()
    y8_out = nc.dram_tensor("y8", [128, F], fp32, kind="ExternalOutput").ap()

    ctx = ExitStack()
    sb = lambda name, shape: ctx.enter_context(
        nc.sbuf_tensor(name, shape, fp32)).ap()
    pst = lambda name: ctx.enter_context(
        nc.psum_tensor(name, [128, F], fp32)).ap()

    cin_t = sb("cin_t", [128, 3 * F])
    win_t = sb("win_t", [128, 384])
    ab_t = cin_t[:, F:2 * F]
    a8_t = cin_t[:, 2 * F:3 * F]
    wden_t = win_t[:, 0:128]
    wk1_t = win_t[:, 128:256]
    wk2_t = win_t[:, 256:384]
    x_t = [sb("x0_t", [128, F]), sb("x1_t", [128, F])]
    xs_t = sb("xs_t", [128, F])
    z_t = [sb("z0_t", [128, F]), sb("z1_t", [128, F])]
    t_t = sb("t_t", [128, F])
    v_t = [sb("v0_t", [128, F]), sb("v1_t", [128, F])]
    idr_t = sb("idr_t", [128, F])
    ln_t = sb("ln_t", [128, F])
    q_t = sb("q_t", [128, F])
    r_t = sb("r_t", [128, F])
    den_ps = pst("den_ps")
    k8_ps = pst("k8_ps")
    ksum_ps = pst("ksum_ps")

    p = _Prog()
    p.op("sync", lambda e: e.dma_start(out=cin_t, in_=cin_in),
         reads=(), writes=["x0raw", "ab", "a8"], inc=16)
    p.op("sync", lambda e: e.dma_start(out=win_t, in_=win_in),
         reads=(), writes=["wden", "wk1", "wk2"], inc=16)
    # state must live in its own ping-pong buffer: copy out of cin
    p.op("vector", lambda e: e.tensor_copy(x_t[0], cin_t[:, 0:F]),
         reads=["x0raw"], writes=["x0"])

    stage_c = [0.5, 0.5, 1.0]
    ksum_w = [("wk1", None), ("wk2", None), ("wk2", None), ("wk1", None)]
    wmap = {"wk1": wk1_t, "wk2": wk2_t}

    for step in range(nsteps):
        xb = f"x{step % 2}"
        xnb = f"x{(step + 1) % 2}"
        x_ap = x_t[step % 2]
        xn_ap = x_t[(step + 1) % 2]
        for s in range(4):
            cur_b = xb if s == 0 else "xs"
            cur_ap = x_ap if s == 0 else xs_t
            zb = s % 2
            vb = s % 2
            p.op("vector",
                 lambda e, o=z_t[zb], a=cur_ap: e.tensor_tensor(o, a, ab_t, Alu.mult),
                 reads=[cur_b, "ab"], writes=[f"z{zb}"])
            p.op("tensor",
                 lambda e, z=z_t[zb]: e.matmul(den_ps, wden_t, z, start=True, stop=True),
                 reads=[f"z{zb}", "wden"], writes=["den"])
            p.op("vector",
                 lambda e, a=cur_ap: e.tensor_tensor(t_t, a, a8_t, Alu.mult),
                 reads=[cur_b, "a8"], writes=["t"])
            if RECIP_MODE == "exact":
                p.op("vector", lambda e: e.reciprocal(idr_t, den_ps),
                     reads=["den"], writes=["idr"])
            else:
                p.op("scalar", lambda e: e.activation(ln_t, den_ps, Act.Ln),
                     reads=["den"], writes=["ln"])
                p.op("scalar",
                     lambda e: e.activation(q_t, ln_t, Act.Exp, scale=-1.0),
                     reads=["ln"], writes=["id0"])
                # one Newton pass: idr = id0 * (2 - den*id0)
                p.op("vector", lambda e: e.tensor_tensor(r_t, den_ps, q_t, Alu.mult),
                     reads=["den", "id0"], writes=["r"])
                p.op("vector",
                     lambda e: e.tensor_scalar(r_t, r_t, -1.0, 2.0, Alu.mult, Alu.add),
                     reads=["r"], writes=["r"])
                p.op("vector", lambda e: e.tensor_tensor(idr_t, q_t, r_t, Alu.mult),
                     reads=["id0", "r"], writes=["idr"])
            p.op("vector",
                 lambda e, o=v_t[vb]: e.tensor_tensor(o, t_t, idr_t, Alu.mult),
                 reads=["t", "idr"], writes=[f"v{vb}"])
            if s < 3:
                p.op("tensor",
                     lambda e, v=v_t[vb]: e.matmul(k8_ps, wk1_t, v, start=True, stop=True),
                     reads=[f"v{vb}", "wk1"], writes=["k8"])
            kw = ksum_w[s][0]
            p.op("tensor",
                 lambda e, w=wmap[kw], v=v_t[vb], st=(s == 0), sp=(s == 3): e.matmul(
                     ksum_ps, w, v, start=st, stop=sp),
                 reads=[f"v{vb}", kw] + ([] if s == 0 else ["ksum"]),
                 writes=["ksum"])
            if s < 3:
                p.op("vector",
                     lambda e, c=stage_c[s], x=x_ap: e.scalar_tensor_tensor(
                         xs_t, k8_ps, float(c), x, Alu.mult, Alu.add),
                     reads=["k8", xb], writes=["xs"])
        p.op("vector",
             lambda e, x=x_ap, xn=xn_ap: e.scalar_tensor_tensor(
                 xn, ksum_ps, 1.0 / 6.0, x, Alu.mult, Alu.add),
             reads=["ksum", xb], writes=[xnb])

    xfb = f"x{nsteps % 2}"
    p.op("sync", lambda e: e.dma_start(out=y8_out, in_=x_t[nsteps % 2]),
         reads=[xfb], writes=["out"], inc=16)

    streams, counts = p.streams()

    with nc.Block() as block, ExitStack() as semctx:
        sems = {n: semctx.enter_context(nc.semaphore(f"sem_{n}"))
                for n in streams}

        def make_body(ename):
            stream = streams[ename]
            total = counts.get(ename, 0)

            def body(eng):
                for waits, o in stream:
                    # fold one wait inline onto the instruction (each
                    # encoding has exactly one sync-wait slot); extras go
                    # on standalone wait_ge instructions before it.
                    for (peng, thr) in waits[:-1]:
                        eng.wait_ge(sems[peng], thr)
                    inst = o.fn(eng)
                    if waits:
                        peng, thr = waits[-1]
                        inst._wait_ge(sems[peng], thr)
                    if o.need_inc:
                        inst.then_inc(sems[o.engine], o.inc)
                if ename == "sync":
                    eng.wait_ge(sems["sync"], total)
            return body

        for ename in streams:
            getattr(block, ename)(make_body(ename))

    ctx.close()
    return nc


_CACHE = {}


def _get_nc():
    if "nc" not in _CACHE:
        _CACHE["nc"] = _build_nc()
    return _CACHE["nc"]


def kernel(y, w, c, t, deltaT):
    from concourse.bass_utils import run_bass_kernel_spmd

    y = np.asarray(y, dtype=np.float32)
    w_np = np.asarray(w, dtype=np.float32)
    c_np = np.asarray(c, dtype=np.float32)
    t_f = np.float32(np.asarray(t))
    dT = np.float32(np.asarray(deltaT))
    h = float(dT) / NSTEPS

    wden, wk1, wk2 = _build_weights()
    win = np.ascontiguousarray(np.concatenate([wden, wk1, wk2], axis=1))
    in_maps = []
    for k in range(NCORES):
        sl = slice(k * PER_CORE, (k + 1) * PER_CORE)
        X8, AB, A8 = _build_core_inputs(y[sl], c_np[sl], h)
        in_maps.append({
            "cin": np.ascontiguousarray(np.concatenate([X8, AB, A8], axis=1)),
            "win": win,
        })

    nc = _get_nc()
    res = run_bass_kernel_spmd(nc, in_maps, list(range(NCORES)))
    y_new = np.concatenate(
        [_extract_y(res.results[k]["y8"]) for k in range(NCORES)], axis=0)
    return (y_new, w_np, c_np, np.float32(t_f + dT))


# revision 8
# speedup vs baseline: 2.1412x; 1.3594x over previous
"""Trainium2 Bass kernel for batched MAPK-cascade ODE step (nn_ModelStep_57715770523995).

Integrates dy/dt = S @ v(y; c) for B=32768 independent trajectories from t to
t+deltaT with fixed-step RK4 (NSTEPS steps), sharded batch-parallel across 8
NeuronCores (4096 trajectories per core).

Per-core layout: [128 partitions, 256 free] = 16 trajectory groups x 8 state
slots x 256 trajectories. Slots hold replicated species [M, Y, T, 1, Y, T, P, M]
so that:
  - den1/den2 (shared Michaelis-Menten denominators) are per-group partition
    sums of slot-wise products -> one block-diagonal fp32 matmul on the tensor
    engine (the "+1" rides in the constant slot 3),
  - the 7 reaction rates are slot-wise products x_s * a_s * (1/den_s) on the
    vector engine,
  - dy = S@v is another block-diagonal +-1 matmul; RK4 stage combinations are
    scalar_tensor_tensor ops.

Raw bass blocks with explicit semaphores (one wait per instruction - the
toolchain's TT/DMA instruction encodings hold a single sync wait).
"""

from collections import defaultdict
from contextlib import ExitStack

import numpy as np

NCORES = 8
B = 32768
PER_CORE = B // NCORES          # 4096
G = 16                          # trajectory groups per core
SLOTS = 8
F = PER_CORE // G               # 256 trajectories per group
NSTEPS = 3
RECIP_MODE = "exact"            # "exact" (DVE reciprocal) | "nr" (exp/ln + 1 Newton)

# slot -> y column (M=0, Y=1, P=2, T=3); -1 means constant 1.0
SLOT_SRC = [0, 1, 3, -1, 1, 3, 2, 0]


# --------------------------------------------------------------------------
# host-side constants / weights
# --------------------------------------------------------------------------
def _derived_constants(c):
    MEK, MKP3, cell = c[:, 0], c[:, 1], c[:, 17]
    d = {}
    d["r2"] = (c[:, 2] + c[:, 6]) / (c[:, 2] * c[:, 6])
    d["ic4"] = 1.0 / c[:, 4]
    d["ic8"] = 1.0 / c[:, 8]
    d["ic10"] = 1.0 / c[:, 10]
    d["ic12"] = 1.0 / c[:, 12]
    d["ic14"] = 1.0 / c[:, 14]
    d["ic16"] = 1.0 / c[:, 16]
    d["a1"] = cell * c[:, 3] * MEK / c[:, 2]
    d["a2"] = cell * c[:, 5] * MEK / c[:, 4]
    d["a3"] = cell * c[:, 7] * MEK / c[:, 6]
    d["a4"] = cell * c[:, 9] * MEK / c[:, 8]
    d["a5"] = cell * c[:, 11] * MKP3 / c[:, 10]
    d["a6"] = cell * c[:, 13] * MKP3 / c[:, 12]
    d["a7"] = cell * c[:, 15] * MKP3 / c[:, 14]
    return d


def _build_core_inputs(yk, ck, h):
    """yk [4096,4] f32, ck [4096,18] f32 -> X8, AB, A8 as [128, F] f32."""
    d = _derived_constants(ck.astype(np.float64))

    def gf(a):
        return a.reshape(G, F)

    y_g = yk.reshape(G, F, 4)
    X8 = np.empty((G, SLOTS, F), np.float32)
    for s, src in enumerate(SLOT_SRC):
        X8[:, s, :] = 1.0 if src < 0 else y_g[:, :, src]
    ab_slots = [d["r2"], d["ic4"], d["ic8"], None,
                d["ic14"], d["ic12"], d["ic10"], d["ic16"]]
    AB = np.empty((G, SLOTS, F), np.float32)
    for s, v in enumerate(ab_slots):
        AB[:, s, :] = 1.0 if v is None else gf(v).astype(np.float32)
    a_slots = [d["a1"], d["a2"], d["a4"], None, d["a7"], d["a6"], d["a5"], d["a3"]]
    A8 = np.empty((G, SLOTS, F), np.float32)
    for s, v in enumerate(a_slots):
        A8[:, s, :] = 0.0 if v is None else (h * gf(v)).astype(np.float32)
    return X8.reshape(128, F), AB.reshape(128, F), A8.reshape(128, F)


def _build_weights():
    """W[k, m] = coefficient of input partition k for output partition m (lhsT)."""
    wden_blk = np.zeros((SLOTS, SLOTS), np.float32)
    for m in (0, 1, 2, 3, 7):            # den1 consumers
        for k in (0, 1, 2, 3):           # den1 = Z0+Z1+Z2+Z3 (Z3 == 1)
            wden_blk[k, m] = 1.0
    for m in (4, 5, 6):                  # den2 consumers
        for k in (3, 4, 5, 6, 7):        # den2 = Z4+..+Z7 + Z3(=1)
            wden_blk[k, m] = 1.0
    # V slots [v1, v2, v4, 0, v7, v6, v5, v3]; out roles [dM,dY,dT,0,dY,dT,dP,dM]
    dy_coef = {
        "dM": {0: -1, 7: -1, 5: +1, 4: +1},
        "dY": {0: +1, 1: -1, 4: -1},
        "dP": {1: +1, 2: +1, 6: -1},
        "dT": {7: +1, 2: -1, 6: +1, 5: -1},
    }
    roles = ["dM", "dY", "dT", None, "dY", "dT", "dP", "dM"]
    wk_blk = np.zeros((SLOTS, SLOTS), np.float32)
    for m, role in enumerate(roles):
        if role is None:
            continue
        for k, coef in dy_coef[role].items():
            wk_blk[k, m] = coef

    def blockdiag(blk):
        W = np.zeros((128, 128), np.float32)
        for g in range(G):
            W[g * 8:(g + 1) * 8, g * 8:(g + 1) * 8] = blk
        return W

    return blockdiag(wden_blk), blockdiag(wk_blk), blockdiag(2.0 * wk_blk)


def _extract_y(y8):
    """y8 [128, F] -> yk [4096, 4]"""
    a = y8.reshape(G, SLOTS, F)
    out = np.empty((G, F, 4), y8.dtype)
    out[:, :, 0] = a[:, 0, :]
    out[:, :, 1] = a[:, 1, :]
    out[:, :, 2] = a[:, 6, :]
    out[:, :, 3] = a[:, 2, :]
    return out.reshape(PER_CORE, 4)


# --------------------------------------------------------------------------
# raw-bass multi-engine scheduler (one sync-wait per instruction)
# --------------------------------------------------------------------------
class _Op:
    __slots__ = ("engine", "fn", "reads", "writes", "inc", "deps",
                 "need_inc", "inc_count")

    def __init__(self, engine, fn, reads, writes, inc):
        self.engine = engine
        self.fn = fn
        self.reads = list(reads)
        self.writes = list(writes)
        self.inc = inc
        self.deps = []
        self.need_inc = False
        self.inc_count = None


class _Prog:
    def __init__(self):
        self.ops = []
        self.writer = {}
        self.readers = defaultdict(list)

    def op(self, engine, fn, reads=(), writes=(), inc=1):
        o = _Op(engine, fn, reads, writes, inc)
        for b in o.reads:
            w = self.writer.get(b)
            if w is not None:
                o.deps.append(w)
        for b in o.writes:
            w = self.writer.get(b)
            if w is not None:
                o.deps.append(w)          # WAW
            for r in self.readers[b]:
                o.deps.append(r)          # WAR
        for b in o.reads:
            self.readers[b].append(o)
        for b in o.writes:
            self.writer[b] = o
            self.readers[b] = []
        self.ops.append(o)
        return o

    def streams(self):
        for o in self.ops:
            if o.engine == "sync":
                o.need_inc = True
            for d in o.deps:
                if d.engine != o.engine:
                    d.need_inc = True
        counts = defaultdict(int)
        for o in self.ops:
            if o.need_inc:
                counts[o.engine] += o.inc
            o.inc_count = counts[o.engine]
        streams = defaultdict(list)
        waited = defaultdict(lambda: defaultdict(int))
        for o in self.ops:
            waits = []
            for d in o.deps:
                if d.engine == o.engine:
                    continue
                thr = d.inc_count
                if waited[o.engine][d.engine] < thr:
                    waits.append((d.engine, thr))
                    waited[o.engine][d.engine] = thr
            streams[o.engine].append((waits, o))
        return streams, dict(counts)


# --------------------------------------------------------------------------
# bass program
# --------------------------------------------------------------------------
def _build_nc(nsteps=None):
    if nsteps is None:
        nsteps = NSTEPS
    import concourse.bass as bass
    import concourse.mybir as mybir

    fp32 = mybir.dt.float32
    Alu = mybir.AluOpType
    Act = mybir.ActivationFunctionType

    nc = bass.Bass(
        "TRN2",
        target_bir_lowering=False,
        debug=False,
        enable_asserts=False,
        num_devices=NCORES,
    )

    cin_in = nc.dram_tensor("cin", [128, 3 * F], fp32, kind="ExternalInput").ap()
    win_in = nc.dram_tensor("win", [128, 256], fp32, kind="ExternalInput").ap()
    y8_out = nc.dram_tensor("y8", [128, F], fp32, kind="ExternalOutput").ap()

    ctx = ExitStack()

    def sbh(name, shape):
        h = ctx.enter_context(nc.sbuf_tensor(name, shape, fp32))
        return h, h.ap()

    sb = lambda name, shape: sbh(name, shape)[1]
    pst = lambda name: ctx.enter_context(
        nc.psum_tensor(name, [128, F], fp32)).ap()

    cin_t = sb("cin_t", [128, 3 * F])
    win_t = sb("win_t", [128, 256])
    aba8_t = cin_t[:, F:3 * F]          # [AB | A8], contiguous in cin
    wden_t = win_t[:, 0:128]
    wk1_t = win_t[:, 128:256]
    xh = [sbh("x0_t", [128, F]), sbh("x1_t", [128, F])]
    xsh = sbh("xs_t", [128, F])
    zt_t = [sb("zt0_t", [128, 512]), sb("zt1_t", [128, 512])]
    v_t = sb("v_t", [128, F])
    vs_t = sb("vs_t", [128, F])
    idr_t = sb("idr_t", [128, F])
    den_ps = pst("den_ps")
    k8_ps = pst("k8_ps")
    ksum_ps = pst("ksum_ps")

    def bcast2(handle):
        # read a [128, F] tile twice along free: [128, 2F] view
        return bass.AP(handle, 0, [[F, 128], [0, 2], [1, F]])

    p = _Prog()
    p.op("sync", lambda e: e.dma_start(out=cin_t, in_=cin_in),
         reads=(), writes=["x0raw", "aba8"], inc=16)
    p.op("sync", lambda e: e.dma_start(out=win_t, in_=win_in),
         reads=(), writes=["wden", "wk1"], inc=16)
    p.op("vector", lambda e: e.tensor_copy(xh[0][1], cin_t[:, 0:F]),
         reads=["x0raw"], writes=["x0"])

    stage_c = [0.5, 0.5, 1.0]
    stage_b = [None, 2.0, 2.0, 1.0]     # VS = V0 + 2*V1 + 2*V2 + V3

    for step in range(nsteps):
        xb = f"x{step % 2}"
        xnb = f"x{(step + 1) % 2}"
        x_h, x_ap = xh[step % 2]
        _, xn_ap = xh[(step + 1) % 2]
        for s in range(4):
            cur_b = xb if s == 0 else "xs"
            cur_h = x_h if s == 0 else xsh[0]
            zb = s % 2
            # fused Z|T: [128,512] = bcast2(cur) * [AB|A8]
            p.op("vector",
                 lambda e, o=zt_t[zb], a=cur_h: e.tensor_tensor(
                     o, bcast2(a), aba8_t, Alu.mult),
                 reads=[cur_b, "aba8"], writes=[f"zt{zb}"])
            p.op("tensor",
                 lambda e, z=zt_t[zb]: e.matmul(
                     den_ps, wden_t, z[:, 0:F], start=True, stop=True),
                 reads=[f"zt{zb}", "wden"], writes=["den"])
            p.op("vector", lambda e: e.reciprocal(idr_t, den_ps),
                 reads=["den"], writes=["idr"])
            vbuf, vname = (vs_t, "vs") if s == 0 else (v_t, "v")
            p.op("vector",
                 lambda e, o=vbuf, z=zt_t[zb]: e.tensor_tensor(
                     o, z[:, F:2 * F], idr_t, Alu.mult),
                 reads=[f"zt{zb}", "idr"], writes=[vname])
            if s > 0:
                p.op("vector",
                     lambda e, b=stage_b[s]: e.scalar_tensor_tensor(
                         vs_t, v_t, float(b), vs_t, Alu.mult, Alu.add),
                     reads=["v", "vs"], writes=["vs"])
            if s < 3:
                p.op("tensor",
                     lambda e, v=vbuf: e.matmul(
                         k8_ps, wk1_t, v, start=True, stop=True),
                     reads=[vname, "wk1"], writes=["k8"])
                p.op("vector",
                     lambda e, c=stage_c[s], x=x_ap: e.scalar_tensor_tensor(
                         xsh[1], k8_ps, float(c), x, Alu.mult, Alu.add),
                     reads=["k8", xb], writes=["xs"])
            else:
                p.op("tensor",
                     lambda e: e.matmul(
                         ksum_ps, wk1_t, vs_t, start=True, stop=True),
                     reads=["vs", "wk1"], writes=["ksum"])
                p.op("vector",
                     lambda e, x=x_ap, xn=xn_ap: e.scalar_tensor_tensor(
                         xn, ksum_ps, 1.0 / 6.0, x, Alu.mult, Alu.add),
                     reads=["ksum", xb], writes=[xnb])

    xfb = f"x{nsteps % 2}"
    p.op("sync", lambda e: e.dma_start(out=y8_out, in_=xh[nsteps % 2][1]),
         reads=[xfb], writes=["out"], inc=16)

    streams, counts = p.streams()

    with nc.Block() as block, ExitStack() as semctx:
        sems = {n: semctx.enter_context(nc.semaphore(f"sem_{n}"))
                for n in streams}

        def make_body(ename):
            stream = streams[ename]
            total = counts.get(ename, 0)

            def body(eng):
                for waits, o in stream:
                    # fold one wait inline onto the instruction (each
                    # encoding has exactly one sync-wait slot); extras go
                    # on standalone wait_ge instructions before it.
                    for (peng, thr) in waits[:-1]:
                        eng.wait_ge(sems[peng], thr)
                    inst = o.fn(eng)
                    if waits:
                        peng, thr = waits[-1]
                        inst._wait_ge(sems[peng], thr)
                    if o.need_inc:
                        inst.then_inc(sems[o.engine], o.inc)
                if ename == "sync":
                    eng.wait_ge(sems["sync"], total)
            return body

        for ename in streams:
            getattr(block, ename)(make_body(ename))

    ctx.close()
    return nc


_CACHE = {}


def _get_nc():
    if "nc" not in _CACHE:
        _CACHE["nc"] = _build_nc()
    return _CACHE["nc"]


def kernel(y, w, c, t, deltaT):
    from concourse.bass_utils import run_bass_kernel_spmd

    y = np.asarray(y, dtype=np.float32)
    w_np = np.asarray(w, dtype=np.float32)
    c_np = np.asarray(c, dtype=np.float32)
    t_f = np.float32(np.asarray(t))
    dT = np.float32(np.asarray(deltaT))
    h = float(dT) / NSTEPS

    wden, wk1, _ = _build_weights()
    win = np.ascontiguousarray(np.concatenate([wden, wk1], axis=1))
    in_maps = []
    for k in range(NCORES):
        sl = slice(k * PER_CORE, (k + 1) * PER_CORE)
        X8, AB, A8 = _build_core_inputs(y[sl], c_np[sl], h)
        in_maps.append({
            "cin": np.ascontiguousarray(np.concatenate([X8, AB, A8], axis=1)),
            "win": win,
        })

    nc = _get_nc()
    res = run_bass_kernel_spmd(nc, in_maps, list(range(NCORES)))
    y_new = np.concatenate(
        [_extract_y(res.results[k]["y8"]) for k in range(NCORES)], axis=0)
    return (y_new, w_np, c_np, np.float32(t_f + dT))


# revision 10
# speedup vs baseline: 6.7633x; 3.1587x over previous
"""Trainium2 Bass kernel for batched MAPK-cascade ODE step (nn_ModelStep_57715770523995).

Integrates dy/dt = S @ v(y; c) for B=32768 independent trajectories from t to
t+deltaT with fixed-step RK4 (NSTEPS steps), sharded batch-parallel across 8
NeuronCores (4096 trajectories per core).

Per-core layout: [128 partitions, 256 free] = 16 trajectory groups x 8 state
slots x 256 trajectories. Slots hold replicated species [M, Y, T, 1, Y, T, P, M]
so that:
  - den1/den2 (shared Michaelis-Menten denominators) are per-group partition
    sums of slot-wise products -> one block-diagonal fp32 matmul on the tensor
    engine (the "+1" rides in the constant slot 3),
  - the 7 reaction rates are slot-wise products x_s * a_s * (1/den_s) on the
    vector engine; the two slot-wise multiplies (den terms and rate
    numerators) are fused into ONE [128,512] op via a stride-0 broadcast AP
    that reads the state twice against the adjacent [AB|A8] constants,
  - dy = S@v is another block-diagonal +-1 matmul; RK4 stage combinations are
    scalar_tensor_tensor ops; the classic-RK4 k-sum is accumulated on the
    vector engine (VS = V0 + 2 V1 + 2 V2 + V3) so only one dy-matmul per
    step remains for it.

Raw bass blocks with explicit semaphores. This toolchain's instruction
encodings hold exactly ONE sync wait, and instruction DISPATCH (~12-28 us
per instruction, globally contended across engines) dominates runtime, so
the program minimizes total instruction count: every data-dependency wait
rides inline on its consumer instruction (BassInstruction._wait_ge), and
the whole 3-step RK4 integration is ~86 dispatched instructions.
"""

from collections import defaultdict
from contextlib import ExitStack

import numpy as np

NCORES = 8
B = 32768
PER_CORE = B // NCORES          # 4096
G = 16                          # trajectory groups per core
SLOTS = 8
F = PER_CORE // G               # 256 trajectories per group
NSTEPS = 3
RECIP_MODE = "exact"            # "exact" (DVE reciprocal) | "nr" (exp/ln + 1 Newton)

# slot -> y column (M=0, Y=1, P=2, T=3); -1 means constant 1.0
SLOT_SRC = [0, 1, 3, -1, 1, 3, 2, 0]


# --------------------------------------------------------------------------
# host-side constants / weights
# --------------------------------------------------------------------------
def _derived_constants(c):
    MEK, MKP3, cell = c[:, 0], c[:, 1], c[:, 17]
    d = {}
    d["r2"] = (c[:, 2] + c[:, 6]) / (c[:, 2] * c[:, 6])
    d["ic4"] = 1.0 / c[:, 4]
    d["ic8"] = 1.0 / c[:, 8]
    d["ic10"] = 1.0 / c[:, 10]
    d["ic12"] = 1.0 / c[:, 12]
    d["ic14"] = 1.0 / c[:, 14]
    d["ic16"] = 1.0 / c[:, 16]
    d["a1"] = cell * c[:, 3] * MEK / c[:, 2]
    d["a2"] = cell * c[:, 5] * MEK / c[:, 4]
    d["a3"] = cell * c[:, 7] * MEK / c[:, 6]
    d["a4"] = cell * c[:, 9] * MEK / c[:, 8]
    d["a5"] = cell * c[:, 11] * MKP3 / c[:, 10]
    d["a6"] = cell * c[:, 13] * MKP3 / c[:, 12]
    d["a7"] = cell * c[:, 15] * MKP3 / c[:, 14]
    return d


def _build_core_inputs(yk, ck, h):
    """yk [4096,4] f32, ck [4096,18] f32 -> X8, AB, A8 as [128, F] f32."""
    d = _derived_constants(ck.astype(np.float64))

    def gf(a):
        return a.reshape(G, F)

    y_g = yk.reshape(G, F, 4)
    X8 = np.empty((G, SLOTS, F), np.float32)
    for s, src in enumerate(SLOT_SRC):
        X8[:, s, :] = 1.0 if src < 0 else y_g[:, :, src]
    ab_slots = [d["r2"], d["ic4"], d["ic8"], None,
                d["ic14"], d["ic12"], d["ic10"], d["ic16"]]
    AB = np.empty((G, SLOTS, F), np.float32)
    for s, v in enumerate(ab_slots):
        AB[:, s, :] = 1.0 if v is None else gf(v).astype(np.float32)
    a_slots = [d["a1"], d["a2"], d["a4"], None, d["a7"], d["a6"], d["a5"], d["a3"]]
    A8 = np.empty((G, SLOTS, F), np.float32)
    for s, v in enumerate(a_slots):
        A8[:, s, :] = 0.0 if v is None else (h * gf(v)).astype(np.float32)
    return X8.reshape(128, F), AB.reshape(128, F), A8.reshape(128, F)


def _build_weights():
    """W[k, m] = coefficient of input partition k for output partition m (lhsT)."""
    wden_blk = np.zeros((SLOTS, SLOTS), np.float32)
    for m in (0, 1, 2, 3, 7):            # den1 consumers
        for k in (0, 1, 2, 3):           # den1 = Z0+Z1+Z2+Z3 (Z3 == 1)
            wden_blk[k, m] = 1.0
    for m in (4, 5, 6):                  # den2 consumers
        for k in (3, 4, 5, 6, 7):        # den2 = Z4+..+Z7 + Z3(=1)
            wden_blk[k, m] = 1.0
    # V slots [v1, v2, v4, 0, v7, v6, v5, v3]; out roles [dM,dY,dT,0,dY,dT,dP,dM]
    dy_coef = {
        "dM": {0: -1, 7: -1, 5: +1, 4: +1},
        "dY": {0: +1, 1: -1, 4: -1},
        "dP": {1: +1, 2: +1, 6: -1},
        "dT": {7: +1, 2: -1, 6: +1, 5: -1},
    }
    roles = ["dM", "dY", "dT", None, "dY", "dT", "dP", "dM"]
    wk_blk = np.zeros((SLOTS, SLOTS), np.float32)
    for m, role in enumerate(roles):
        if role is None:
            continue
        for k, coef in dy_coef[role].items():
            wk_blk[k, m] = coef

    def blockdiag(blk):
        W = np.zeros((128, 128), np.float32)
        for g in range(G):
            W[g * 8:(g + 1) * 8, g * 8:(g + 1) * 8] = blk
        return W

    return blockdiag(wden_blk), blockdiag(wk_blk), blockdiag(2.0 * wk_blk)


def _extract_y(y8):
    """y8 [128, F] -> yk [4096, 4]"""
    a = y8.reshape(G, SLOTS, F)
    out = np.empty((G, F, 4), y8.dtype)
    out[:, :, 0] = a[:, 0, :]
    out[:, :, 1] = a[:, 1, :]
    out[:, :, 2] = a[:, 6, :]
    out[:, :, 3] = a[:, 2, :]
    return out.reshape(PER_CORE, 4)


# --------------------------------------------------------------------------
# raw-bass multi-engine scheduler (one sync-wait per instruction)
# --------------------------------------------------------------------------
class _Op:
    __slots__ = ("engine", "fn", "reads", "writes", "inc", "deps",
                 "need_inc", "inc_count")

    def __init__(self, engine, fn, reads, writes, inc):
        self.engine = engine
        self.fn = fn
        self.reads = list(reads)
        self.writes = list(writes)
        self.inc = inc
        self.deps = []
        self.need_inc = False
        self.inc_count = None


class _Prog:
    def __init__(self):
        self.ops = []
        self.writer = {}
        self.readers = defaultdict(list)

    def op(self, engine, fn, reads=(), writes=(), inc=1):
        o = _Op(engine, fn, reads, writes, inc)
        for b in o.reads:
            w = self.writer.get(b)
            if w is not None:
                o.deps.append(w)
        for b in o.writes:
            w = self.writer.get(b)
            if w is not None:
                o.deps.append(w)          # WAW
            for r in self.readers[b]:
                o.deps.append(r)          # WAR
        for b in o.reads:
            self.readers[b].append(o)
        for b in o.writes:
            self.writer[b] = o
            self.readers[b] = []
        self.ops.append(o)
        return o

    def streams(self):
        for o in self.ops:
            if o.engine == "sync":
                o.need_inc = True
            for d in o.deps:
                if d.engine != o.engine:
                    d.need_inc = True
        counts = defaultdict(int)
        for o in self.ops:
            if o.need_inc:
                counts[o.engine] += o.inc
            o.inc_count = counts[o.engine]
        streams = defaultdict(list)
        waited = defaultdict(lambda: defaultdict(int))
        for o in self.ops:
            waits = []
            for d in o.deps:
                if d.engine == o.engine:
                    continue
                thr = d.inc_count
                if waited[o.engine][d.engine] < thr:
                    waits.append((d.engine, thr))
                    waited[o.engine][d.engine] = thr
            streams[o.engine].append((waits, o))
        return streams, dict(counts)


# --------------------------------------------------------------------------
# bass program
# --------------------------------------------------------------------------
def _build_nc(nsteps=None):
    if nsteps is None:
        nsteps = NSTEPS
    import concourse.bass as bass
    import concourse.mybir as mybir

    fp32 = mybir.dt.float32
    Alu = mybir.AluOpType
    Act = mybir.ActivationFunctionType

    nc = bass.Bass(
        "TRN2",
        target_bir_lowering=False,
        debug=False,
        enable_asserts=False,
        num_devices=NCORES,
    )

    inp_in = nc.dram_tensor("inp", [128, 4 * F], fp32, kind="ExternalInput").ap()
    y8_out = nc.dram_tensor("y8", [128, F], fp32, kind="ExternalOutput").ap()

    ctx = ExitStack()

    def sbh(name, shape):
        h = ctx.enter_context(nc.sbuf_tensor(name, shape, fp32))
        return h, h.ap()

    sb = lambda name, shape: sbh(name, shape)[1]
    pst = lambda name: ctx.enter_context(
        nc.psum_tensor(name, [128, F], fp32)).ap()

    inp_h, inp_t = sbh("inp_t", [128, 4 * F])
    aba8_t = inp_t[:, F:3 * F]          # [AB | A8], contiguous
    wden_t = inp_t[:, 3 * F:3 * F + 128]
    wk1_t = inp_t[:, 3 * F + 128:4 * F]
    xh = [sbh("x0_t", [128, F]), sbh("x1_t", [128, F])]
    xsh = sbh("xs_t", [128, F])
    zt_t = [sb("zt0_t", [128, 512]), sb("zt1_t", [128, 512])]
    v_t = sb("v_t", [128, F])
    vs_t = sb("vs_t", [128, F])
    idr_t = sb("idr_t", [128, F])
    den_ps = pst("den_ps")
    k8_ps = pst("k8_ps")
    ksum_ps = pst("ksum_ps")

    def bcast2(handle, pstride=F):
        # read a [128, F] tile twice along free: [128, 2F] view
        return bass.AP(handle, 0, [[pstride, 128], [0, 2], [1, F]])

    p = _Prog()
    p.op("sync", lambda e: e.dma_start(out=inp_t, in_=inp_in),
         reads=(), writes=["x_init", "aba8", "wden", "wk1"], inc=16)

    stage_c = [0.5, 0.5, 1.0]
    stage_b = [None, 2.0, 2.0, 1.0]     # VS = V0 + 2*V1 + 2*V2 + V3

    for step in range(nsteps):
        first = step == 0
        xb = "x_init" if first else f"x{step % 2}"
        xnb = f"x{(step + 1) % 2}"
        x_h = inp_h if first else xh[step % 2][0]
        x_ap = inp_t[:, 0:F] if first else xh[step % 2][1]
        x_ps = 4 * F if first else F
        _, xn_ap = xh[(step + 1) % 2]
        for s in range(4):
            cur_b = xb if s == 0 else "xs"
            cur_h = x_h if s == 0 else xsh[0]
            cur_ps = x_ps if s == 0 else F
            zb = s % 2
            # fused Z|T: [128,512] = bcast2(cur) * [AB|A8]
            p.op("vector",
                 lambda e, o=zt_t[zb], a=cur_h, ps=cur_ps: e.tensor_tensor(
                     o, bcast2(a, ps), aba8_t, Alu.mult),
                 reads=[cur_b, "aba8"], writes=[f"zt{zb}"])
            p.op("tensor",
                 lambda e, z=zt_t[zb]: e.matmul(
                     den_ps, wden_t, z[:, 0:F], start=True, stop=True),
                 reads=[f"zt{zb}", "wden"], writes=["den"])
            p.op("vector", lambda e: e.reciprocal(idr_t, den_ps),
                 reads=["den"], writes=["idr"])
            vbuf, vname = (vs_t, "vs") if s == 0 else (v_t, "v")
            p.op("vector",
                 lambda e, o=vbuf, z=zt_t[zb]: e.tensor_tensor(
                     o, z[:, F:2 * F], idr_t, Alu.mult),
                 reads=[f"zt{zb}", "idr"], writes=[vname])
            if s == 3:
                p.op("vector",
                     lambda e, b=stage_b[s]: e.scalar_tensor_tensor(
                         vs_t, v_t, float(b), vs_t, Alu.mult, Alu.add),
                     reads=["v", "vs"], writes=["vs"])
            if s < 3:
                p.op("tensor",
                     lambda e, v=vbuf: e.matmul(
                         k8_ps, wk1_t, v, start=True, stop=True),
                     reads=[vname, "wk1"], writes=["k8"])
                p.op("vector",
                     lambda e, c=stage_c[s], x=x_ap: e.scalar_tensor_tensor(
                         xsh[1], k8_ps, float(c), x, Alu.mult, Alu.add),
                     reads=["k8", xb], writes=["xs"])
                if s > 0:
                    # off-path: accumulate while PE runs the next den-matmul
                    p.op("vector",
                         lambda e, b=stage_b[s]: e.scalar_tensor_tensor(
                             vs_t, v_t, float(b), vs_t, Alu.mult, Alu.add),
                         reads=["v", "vs"], writes=["vs"])
            else:
                p.op("tensor",
                     lambda e: e.matmul(
                         ksum_ps, wk1_t, vs_t, start=True, stop=True),
                     reads=["vs", "wk1"], writes=["ksum"])
                p.op("vector",
                     lambda e, x=x_ap, xn=xn_ap: e.scalar_tensor_tensor(
                         xn, ksum_ps, 1.0 / 6.0, x, Alu.mult, Alu.add),
                     reads=["ksum", xb], writes=[xnb])

    xfb = f"x{nsteps % 2}"
    p.op("sync", lambda e: e.dma_start(out=y8_out, in_=xh[nsteps % 2][1]),
         reads=[xfb], writes=["out"], inc=16)

    streams, counts = p.streams()

    with nc.Block() as block, ExitStack() as semctx:
        sems = {n: semctx.enter_context(nc.semaphore(f"sem_{n}"))
                for n in streams}

        def make_body(ename):
            stream = streams[ename]
            total = counts.get(ename, 0)

            def body(eng):
                for waits, o in stream:
                    # fold one wait inline onto the instruction (each
                    # encoding has exactly one sync-wait slot); extras go
                    # on standalone wait_ge instructions before it.
                    for (peng, thr) in waits[:-1]:
                        eng.wait_ge(sems[peng], thr)
                    inst = o.fn(eng)
                    if waits:
                        peng, thr = waits[-1]
                        inst._wait_ge(sems[peng], thr)
                    if o.need_inc:
                        inst.then_inc(sems[o.engine], o.inc)
                if ename == "sync":
                    eng.wait_ge(sems["sync"], total)
            return body

        for ename in streams:
            getattr(block, ename)(make_body(ename))

    ctx.close()
    return nc


_CACHE = {}


def _get_nc():
    if "nc" not in _CACHE:
        _CACHE["nc"] = _build_nc()
    return _CACHE["nc"]


def kernel(y, w, c, t, deltaT):
    from concourse.bass_utils import run_bass_kernel_spmd

    y = np.asarray(y, dtype=np.float32)
    w_np = np.asarray(w, dtype=np.float32)
    c_np = np.asarray(c, dtype=np.float32)
    t_f = np.float32(np.asarray(t))
    dT = np.float32(np.asarray(deltaT))
    h = float(dT) / NSTEPS

    wden, wk1, _ = _build_weights()
    win = np.concatenate([wden, wk1], axis=1)
    in_maps = []
    for k in range(NCORES):
        sl = slice(k * PER_CORE, (k + 1) * PER_CORE)
        X8, AB, A8 = _build_core_inputs(y[sl], c_np[sl], h)
        in_maps.append({
            "inp": np.ascontiguousarray(
                np.concatenate([X8, AB, A8, win], axis=1)),
        })

    nc = _get_nc()
    res = run_bass_kernel_spmd(nc, in_maps, list(range(NCORES)))
    y_new = np.concatenate(
        [_extract_y(res.results[k]["y8"]) for k in range(NCORES)], axis=0)
    return (y_new, w_np, c_np, np.float32(t_f + dT))
